# revision 3
# baseline (speedup 1.0000x reference)
"""Causal masked single-head attention [B=4, N=4096, D=768] on 8 trn2 cores.

Sharding: 2 cores per batch element; core parity c owns global q-blocks
2j+c (j=0..7, 256 rows each), round-robin so causal work balances. One
identical SPMD instruction stream; per-core differences live in input data
(query gather order, multiplicative causal masks).

Algebraic fold: scores = (X Wq^T)(X Wk^T)^T = X G X^T with G = Wq^T Wk
precomputed on the host. The device projects only YQ = G^T xq (the K
projection disappears — scores contract YQ against raw X). G is scaled by
64 into fp8's normal range; the 1/64 folds into the softmax exp scale.

fp8 (e4m3) DoubleRow matmuls carry the bulk of the FLOPs at 2x+ bf16 PE
throughput: the YQ projection, all scores, and P@V for key tiles >= 4.
Precision floors fp8 can't hit are patched in bf16 where softmax averaging
can't dilute quantization noise: V for the first 512 keys is projected and
stored in bf16, and the j==0 q-block does its P@V in bf16 (covers output
rows whose softmax support is only a handful of keys).

Throughput structure:
- Emission interleaves phases: YQ projection, then per 512-key block kb:
  V projection of kb followed by attention q-block j=kb-1 (one block
  behind, so engine queues never block head-of-line on fresh tiles).
- Scores for a QUAD of key tiles accumulate into one [128,1024] 2-bank PSUM
  tile (4 groups; groups 1,3 ride their bank's pending-zero from groups 0,2
  via start=False + skip_group_check); exp'd in two 512 halves so P@V's
  first half never waits a full-quad exp.
- Causal masking is multiplicative AFTER exp (0/1 mask) on the otherwise
  idle Pool engine (GPSIMD cannot touch PSUM, but pt lives in SBUF).
- V carries a ones-column (col 768) so the softmax denominator falls out of
  the P@V accumulation; P@V output chunks are {512, 257}.
- PSUM: shared ring of [128,1024] tiles (projections + score quads) + ops.

Subtile layout for DoubleRow: operands live as [128, nsub, free] where
subtile t holds contraction rows 128t..128t+127; a DR matmul consumes a
[:, 2t:2t+2, :] slice (256 contraction rows per instruction).
"""

import math
import sys

sys.path.insert(0, "/opt/trn_rl_repo")

import numpy as np
import ml_dtypes

import concourse.bass as bass
import concourse.bacc as bacc
import concourse.mybir as mybir
import concourse.tile as tile
from concourse.bass_utils import run_bass_kernel_spmd

F32 = mybir.dt.float32
BF16 = mybir.dt.bfloat16
F8 = mybir.dt.float8e4
DR = mybir.MatmulPerfMode.DoubleRow

GS = 64.0  # host-side scale on G (into fp8 normal range); folded into exp
VW = 769   # V tile free width: 768 values + ones col at 768
PV0 = 512  # first P@V output chunk
PV1 = VW - PV0  # second chunk (257, denominator at its col 256)


class Cfg:
    def __init__(self, D=768, N=4096, QB=256):
        assert D % 128 == 0 and N % 512 == 0 and QB == 256
        self.D = D
        self.N = N
        self.QB = QB
        self.QC = N // 2
        self.NDT = D // 128   # input-dim subtiles
        self.NOT = D // 128   # output-dim subtiles
        self.NKB = N // 512   # 512-key blocks
        self.NKT = N // 128   # 128-key tiles
        self.NQB = self.QC // QB
        self.scale = 1.0 / math.sqrt(D)


def build_kernel(cfg: Cfg, repeat: int = 1, dynamic_repeat: bool = False) -> bass.Bass:
    """dynamic_repeat: trip count read from an int32 input "rcount" at
    runtime — one executable measures any R (for two-point device timing)."""
    import contextlib
    D, N, QB, QC = cfg.D, cfg.N, cfg.QB, cfg.QC
    NDT, NOT = cfg.NDT, cfg.NOT
    nc = bacc.Bacc("TRN2")
    rc = (nc.dram_tensor("rcount", [1, 1], mybir.dt.int32, kind="ExternalInput")
          if dynamic_repeat else None)

    xT8 = nc.dram_tensor("xT8", [128, NDT, N], F8, kind="ExternalInput")
    xT16 = nc.dram_tensor("xT16", [128, NDT, 256], BF16, kind="ExternalInput")
    xqT8 = nc.dram_tensor("xqT8", [128, NDT, QC], F8, kind="ExternalInput")
    g8 = nc.dram_tensor("g8", [128, NDT, D], F8, kind="ExternalInput")
    wv8 = nc.dram_tensor("wv8", [128, NDT, D], F8, kind="ExternalInput")
    wv16 = nc.dram_tensor("wv16", [128, NDT, D], BF16, kind="ExternalInput")
    mask01 = nc.dram_tensor("mask01", [128, 4 * QB], BF16, kind="ExternalInput")
    # raw P@V accumulators (numerator chunks + denominator col); the host
    # does the softmax division during unshard
    out0 = nc.dram_tensor("out0", [QC, PV0], F32, kind="ExternalOutput")
    out1 = nc.dram_tensor("out1", [QC, PV1], F32, kind="ExternalOutput")

    with tile.TileContext(nc) as tc:
        with (
            tc.tile_pool(name="persist", bufs=1) as persist,
            tc.tile_pool(name="xstream", bufs=2) as xstream,
            tc.tile_pool(name="work", bufs=3) as work,
            tc.tile_pool(name="big", bufs=2, space="PSUM") as bigp,
            tc.tile_pool(name="ops0", bufs=2, space="PSUM") as ops0p,
            tc.tile_pool(name="ops1", bufs=2, space="PSUM") as ops1p,
        ):
            g_t = persist.tile([128, NDT, D], F8, tag="g")
            wv_t = persist.tile([128, NDT, D], F8, tag="wv")
            wv16_t = persist.tile([128, NDT, D], BF16, tag="wv16")
            X8 = [persist.tile([128, NDT, 512], F8, tag=f"X8_{kb}",
                               name=f"X8_{kb}") for kb in range(cfg.NKB)]
            YQ = [persist.tile([128, NOT, 512], F8, tag=f"YQ{qb}", name=f"YQ{qb}")
                  for qb in range(QC // 512)]
            V8 = [persist.tile([128, 2, VW], F8, tag=f"V8_{kp}", name=f"V8_{kp}")
                  for kp in range(cfg.NKT // 2)]
            V16 = [persist.tile([128, VW], BF16, tag=f"V16_{kt}", name=f"V16_{kt}")
                   for kt in range(4)]
            msk = persist.tile([128, 4 * QB], BF16, tag="msk")

            # PSUM->SBUF copies on DVE by default (ScalarE owns the exps);
            # alt=True alternates onto ScalarE too (for the YQ phase, where
            # no exps compete)
            cp_state = [0]

            def psum_copy(dst, src, alt=False):
                cp_state[0] += 1
                if alt and cp_state[0] % 2 == 0:
                    nc.scalar.copy(dst, src)
                else:
                    nc.vector.tensor_copy(dst, src)

            if dynamic_repeat:
                rct = persist.tile([1, 1], mybir.dt.int32, tag="rct")
                nc.sync.dma_start(rct[:], rc[:])
                rval = nc.values_load(rct[:], min_val=1, max_val=1 << 20)
                rep_ctx = tc.For_i(0, rval, 1)
            elif repeat > 1:
                rep_ctx = tc.For_i(0, repeat, 1)
            else:
                rep_ctx = contextlib.nullcontext()
            with rep_ctx:
                # x blocks stream on the ScalarE queue (its first compute
                # comes after the YQ matmuls), xq on sync, weights/mask on
                # Pool — three DMA queues in parallel at startup
                xb16 = persist.tile([128, NDT, 256], BF16, tag="xb16")
                nc.scalar.dma_start(X8[0][:], xT8[:, :, 0:512])
                nc.scalar.dma_start(xb16[:], xT16[:])
                for kb in range(1, 3):
                    nc.scalar.dma_start(X8[kb][:],
                                        xT8[:, :, 512 * kb:512 * (kb + 1)])

                x_loaded = {0, 1, 2}

                def late_x_dma(kb):
                    if 0 <= kb < cfg.NKB and kb not in x_loaded:
                        x_loaded.add(kb)
                        nc.scalar.dma_start(X8[kb][:],
                                            xT8[:, :, 512 * kb:512 * (kb + 1)])
                nc.gpsimd.dma_start(g_t[:], g8[:])
                nc.gpsimd.dma_start(wv_t[:], wv8[:])
                nc.gpsimd.dma_start(wv16_t[:], wv16[:])
                nc.gpsimd.dma_start(msk[:], mask01[:])
                for kp in range(cfg.NKT // 2):
                    nc.gpsimd.memset(V8[kp][:, :, 768:769], 1.0)
                for kt in range(4):
                    nc.gpsimd.memset(V16[kt][:, 768:769], 1.0)

                # ---- YQ = G^T xq projection pieces (fp8); big tiles are
                # [128, 2, 512]: one 512-col bank per subindex. Only qb=0
                # runs up front; qb>=1 interleave into early attention ----
                xq_tiles = {}

                def xq_dma(qb):
                    xq = xstream.tile([128, NDT, 512], F8, tag="xq",
                                      name=f"xq{qb}")
                    nc.sync.dma_start(xq[:], xqT8[:, :, 512 * qb:512 * (qb + 1)])
                    xq_tiles[qb] = xq

                def yq_op(qb, op):
                    ps = bigp.tile([128, 2, 512], F32, tag="big", name="yqproj")
                    for i in range(2):
                        ot = 2 * op + i
                        for t in range(NDT // 2):
                            nc.tensor.matmul(
                                ps[:, i, :],
                                g_t[:, 2 * t:2 * t + 2, 128 * ot:128 * (ot + 1)],
                                xq_tiles[qb][:, 2 * t:2 * t + 2, :],
                                start=(t == 0), stop=(t == NDT // 2 - 1),
                                perf_mode=DR)
                    psum_copy(YQ[qb][:, 2 * op:2 * op + 2, :], ps[:], alt=True)

                def attention(j, fillers=()):
                    """Attention q-block j. All X/V/YQ dependencies are at
                    least one projection block old when this is emitted.
                    fillers: thunks (V-proj kl groups of the NEXT block)
                    emitted between quads so PE fills exp-wait gaps."""
                    fillers = list(fillers)
                    nkt = 4 * j + 4
                    nquad = nkt // 4
                    qb = j // 2
                    qcol = QB * (j % 2)
                    ops0 = [ops0p.tile([128, PV0], F32, tag="ops0",
                                       name=f"o0_{qh}") for qh in range(2)]
                    ops1 = [ops1p.tile([128, PV1], F32, tag="ops1",
                                       name=f"o1_{qh}") for qh in range(2)]

                    def scores_quad(kq):
                        st4 = bigp.tile([128, 2, 512], F32, tag="big", name="st4")
                        for i in range(4):
                            kt = 4 * kq + i
                            for t in range(NOT // 2):
                                nc.tensor.matmul(
                                    st4[:, i // 2, 256 * (i % 2):256 * (i % 2 + 1)],
                                    X8[kt // 4][:, 2 * t:2 * t + 2,
                                                128 * (kt % 4):128 * (kt % 4 + 1)],
                                    YQ[qb][:, 2 * t:2 * t + 2, qcol:qcol + QB],
                                    start=(i % 2 == 0 and t == 0),
                                    stop=(t == NOT // 2 - 1),
                                    perf_mode=DR,
                                    skip_group_check=(i % 2 == 1))
                        if j == 0:
                            pt = work.tile([128, 1024], BF16, tag="pt16",
                                           name="pt16")
                            half = lambda tl, h: tl[:, 512 * h:512 * (h + 1)]
                        else:
                            pt = work.tile([128, 4, QB], F8, tag="pt8",
                                           name="pt8")
                            half = lambda tl, h: tl[:, 2 * h:2 * h + 2, :]
                        # exp in two 512 halves: P@V's h=0 group only needs
                        # half 0, so PE never waits a full-quad exp
                        for h in range(2):
                            nc.scalar.activation(half(pt, h),
                                                 st4[:, h, :],
                                                 mybir.ActivationFunctionType.Exp,
                                                 scale=cfg.scale / GS)
                            if kq == nquad - 1:  # causal tail: 0/1 mask
                                nc.gpsimd.tensor_mul(
                                    half(pt, h), half(pt, h),
                                    msk[:, 512 * h:512 * (h + 1)])
                        return pt

                    def pv(kq, pt, first, last):
                        if j == 0:
                            for kt in range(4):
                                for qh in range(2):
                                    nc.tensor.matmul(
                                        ops0[qh][:],
                                        pt[:, 256 * kt + 128 * qh:
                                           256 * kt + 128 * (qh + 1)],
                                        V16[kt][:, 0:PV0],
                                        start=(kt == 0), stop=(kt == 3))
                                    nc.tensor.matmul(
                                        ops1[qh][:],
                                        pt[:, 256 * kt + 128 * qh:
                                           256 * kt + 128 * (qh + 1)],
                                        V16[kt][:, PV0:VW],
                                        start=(kt == 0), stop=(kt == 3))
                        else:
                            for qh in range(2):
                                for h in range(2):
                                    nc.tensor.matmul(
                                        ops0[qh][:],
                                        pt[:, 2 * h:2 * h + 2,
                                           128 * qh:128 * (qh + 1)],
                                        V8[2 * kq + h][:, :, 0:PV0],
                                        start=(first and h == 0),
                                        stop=(last and h == 1),
                                        perf_mode=DR)
                                    nc.tensor.matmul(
                                        ops1[qh][:],
                                        pt[:, 2 * h:2 * h + 2,
                                           128 * qh:128 * (qh + 1)],
                                        V8[2 * kq + h][:, :, PV0:VW],
                                        start=(first and h == 0),
                                        stop=(last and h == 1),
                                        perf_mode=DR)

                    # process the masked tail quad FIRST so its exp -> mask
                    # -> P@V chain hides under the remaining quads' work
                    order = ([nquad - 1] + list(range(nquad - 1))
                             if nquad > 1 else [0])
                    pts = {order[0]: scores_quad(order[0])}
                    for i, kq in enumerate(order):
                        if i + 1 < nquad:
                            pts[order[i + 1]] = scores_quad(order[i + 1])
                        pv(kq, pts.pop(kq), first=(i == 0), last=(i == nquad - 1))
                        if fillers:
                            fillers.pop(0)()
                    while fillers:
                        fillers.pop(0)()

                    # stage raw accumulators to SBUF (split across ScalarE /
                    # DVE) and DMA out; the host does the softmax division
                    for qh in range(2):
                        r0, r1 = QB * j + 128 * qh, QB * j + 128 * (qh + 1)
                        osb0 = work.tile([128, PV0], F32, tag="osb0", name="osb0")
                        osb1 = work.tile([128, PV1], F32, tag="osb1", name="osb1")
                        nc.scalar.copy(osb0[:], ops0[qh][:])
                        nc.vector.tensor_copy(osb1[:], ops1[qh][:])
                        nc.sync.dma_start(out0[r0:r1, :], osb0[:])
                        nc.sync.dma_start(out1[r0:r1, :], osb1[:])

                def vproj_kl(kb, kl):
                    """One V-projection kl group of block kb: chunk c in bank
                    c of a [128,2,512] tile, one fused copy. Block 0 kl 0,1
                    (keys 0..255, the only keys whose P@V rows can have tiny
                    softmax support) run in bf16; everything else fp8 DR."""
                    kt = 4 * kb + kl
                    ps = bigp.tile([128, 2, 512], F32, tag="big", name="vproj")
                    for c in range(2):
                        o0 = 384 * c
                        if kb == 0 and kl < 2:
                            for dt in range(NDT):
                                nc.tensor.matmul(
                                    ps[:, c, 0:384],
                                    xb16[:, dt, 128 * kl:128 * (kl + 1)],
                                    wv16_t[:, dt, o0:o0 + 384],
                                    start=(dt == 0), stop=(dt == NDT - 1))
                        else:
                            for t in range(NDT // 2):
                                nc.tensor.matmul(
                                    ps[:, c, 0:384],
                                    X8[kb][:, 2 * t:2 * t + 2,
                                           128 * kl:128 * (kl + 1)],
                                    wv_t[:, 2 * t:2 * t + 2, o0:o0 + 384],
                                    start=(t == 0), stop=(t == NDT // 2 - 1),
                                    perf_mode=DR)
                    if kb == 0:
                        psum_copy(V16[kt][:, 0:768], ps[:, :, 0:384])
                    psum_copy(V8[kt // 2][:, kt % 2, 0:768], ps[:, :, 0:384])

                # ---- schedule: YQ(qb0) + V blocks 1,0 up front (0 after 1:
                # the bf16 path waits on wv16/xb16 DMAs); attention(j) then
                # absorbs later YQ blocks and V-proj(j+2) kl groups as PE
                # gap-filler between its quads ----
                xq_dma(0)
                xq_dma(1)
                for op in range(NOT // 2):
                    yq_op(0, op)
                for kl in range(4):
                    vproj_kl(1, kl)
                for kl in range(4):
                    vproj_kl(0, kl)
                for j in range(cfg.NQB):
                    kb_next = j + 2
                    late_x_dma(kb_next + 1)
                    fillers = []
                    if j + 1 < QC // 512:  # YQ block j+1 (used from attn 2j+2)
                        if j + 2 < QC // 512:
                            fillers.append(lambda qb=j + 2: xq_dma(qb))
                        fillers.extend(
                            (lambda qb=j + 1, op=op: yq_op(qb, op))
                            for op in range(NOT // 2))
                    if kb_next < cfg.NKB:
                        fillers.extend(
                            (lambda kb=kb_next, kl=kl: vproj_kl(kb, kl))
                            for kl in range(4))
                    attention(j, fillers)
    nc.compile()
    return nc


# ---------------------------------------------------------------------------
# Host-side sharding / gather
# ---------------------------------------------------------------------------

F8NP = ml_dtypes.float8_e4m3fn
BF = ml_dtypes.bfloat16


def make_mask01(QB: int, parity: int) -> np.ndarray:
    kk = np.arange(128)[:, None]
    qq = np.arange(QB)[None, :]
    tri0 = (kk <= qq).astype(np.float32)
    tri1 = (kk + 128 <= qq).astype(np.float32)
    one = np.ones((128, QB), np.float32)
    zero = np.zeros((128, QB), np.float32)
    blocks = [tri0, tri1, zero, zero] if parity == 0 else [one, one, tri0, tri1]
    return np.concatenate(blocks, axis=1).astype(BF)


def _subtiled(a_T: np.ndarray, ndt: int, dtype) -> np.ndarray:
    """[768, cols] -> [128, ndt, cols] with subtile t = rows 128t..128t+127."""
    cols = a_T.shape[1]
    return np.ascontiguousarray(
        a_T.reshape(ndt, 128, cols).transpose(1, 0, 2)).astype(dtype)


def core_inputs(cfg: Cfg, x_b: np.ndarray, g8s, wv8s, wv16_s, parity: int) -> dict:
    QB = cfg.QB
    xT = x_b.T  # [768, 4096]
    cols = []
    for j in range(cfg.NQB):
        gb = 2 * j + parity
        cols.append(x_b[QB * gb:QB * (gb + 1), :].T)
    xqT = np.concatenate(cols, axis=1)  # [768, 2048]
    return {
        "xT8": _subtiled(xT, cfg.NDT, F8NP),
        "xT16": _subtiled(xT[:, :256], cfg.NDT, BF),
        "xqT8": _subtiled(xqT, cfg.NDT, F8NP),
        "g8": g8s,
        "wv8": wv8s,
        "wv16": wv16_s,
        "mask01": make_mask01(QB, parity),
    }


def scatter_output(cfg: Cfg, out0: np.ndarray, out1: np.ndarray, parity: int,
                   dst: np.ndarray) -> None:
    """Normalize raw accumulators (num/den) and scatter rows to global order."""
    QB = cfg.QB
    den = out1[:, PV1 - 1:PV1]
    num = np.concatenate([out0, out1[:, :PV1 - 1]], axis=1)
    o = num / den
    for j in range(cfg.NQB):
        gb = 2 * j + parity
        dst[QB * gb:QB * (gb + 1), :] = o[QB * j:QB * (j + 1), :]


def build_in_maps(cfg: Cfg, input_batch, Wq, Wk, Wv):
    x = np.asarray(input_batch, dtype=np.float32)
    Wq = np.asarray(Wq, np.float32)
    Wk = np.asarray(Wk, np.float32)
    Wv = np.asarray(Wv, np.float32)
    G = (Wq.T @ Wk) * GS  # scores = x G x^T; GS folds into exp scale
    g8s = _subtiled(G, cfg.NDT, F8NP)
    wv8s = _subtiled(Wv.T, cfg.NDT, F8NP)
    wv16_s = _subtiled(Wv.T, cfg.NDT, BF)
    return [core_inputs(cfg, x[c // 2], g8s, wv8s, wv16_s, c % 2)
            for c in range(8)]


_CACHE: dict = {}


def _get_nc(cfg: Cfg) -> bass.Bass:
    if "nc" not in _CACHE:
        _CACHE["nc"] = build_kernel(cfg)
    return _CACHE["nc"]


def kernel(input_batch, Wq, Wk, Wv):
    cfg = Cfg()
    nc = _get_nc(cfg)
    in_maps = build_in_maps(cfg, input_batch, Wq, Wk, Wv)
    res = run_bass_kernel_spmd(nc, in_maps, core_ids=list(range(8)))
    B = np.asarray(input_batch).shape[0]
    out = np.empty((B, cfg.N, cfg.D), np.float32)
    for c in range(2 * B):
        scatter_output(cfg, res.results[c]["out0"], res.results[c]["out1"],
                       c % 2, out[c // 2])
    return out
